# revision 18
# baseline (speedup 1.0000x reference)
"""Trainium2 Bass kernel for nn_AdaptiveConditionedGraphTransformer.

Strategy (8 NeuronCores, data-parallel over nodes, 128 nodes/core):
- Graph edge-attention (PyG TransformerConv) computed DENSELY per core as
  [src=1024, dst=128] score matrices on TensorE; duplicate edges + masking
  handled exactly via a host-precomputed ln(count) additive mask injected
  into PSUM with an identity matmul inside the QK^T accumulation group.
- Layer-0 tconv (fin=10) uses a rank-11 factorization: S = x_aug A x_aug^T
  with A = Wk_aug Wq_aug^T/16 precomputed on host -> no K/V materialization
  and only a 5.5KB/rank AllGather of x_aug per autoregressive step.
- Layer-1 tconv computes K^T/V locally; the 10 independent known steps share
  ONE batched 5MB AllGather; autoregressive steps use split K/V AllGathers
  so score matmuls overlap the V gather.
- Softmax denominators ride along in the aggregation matmuls (ones column
  appended to V / e10 column appended to Wv0aug) -- no separate z matmuls.
- Temporal transformer layer: exact KV-cache, last-query only, on VectorE.
- LayerNorm rsqrt via exp(-0.5*ln(var+eps)) so ScalarE stays on the single
  natural_log_exp table set the whole kernel.
- All matmul inputs fp16 (fp32 PSUM accumulate); softmax/LN math fp32.

kernel(**inputs) takes FULL inputs, shards internally, returns FULL output.
"""
import os
import sys

import numpy as np

sys.path.insert(0, "/opt/trn_rl_repo")

N, E, F = 1024, 16384, 10
DM, H = 256, 4
D = 256
NG, K = 20, 10
FF = 2048
DH = DM // H
NC_ = 8
P = N // NC_          # 128 nodes per core
AUG = F + 1           # 11
TCACHE = NG - 1       # 19 cache slots

N_KNOWN = int(os.environ.get("GT_KNOWN", "10"))
N_GEN = int(os.environ.get("GT_GEN", "10"))

_NEG = -30000.0


# ----------------------------------------------------------------------------
# Device program
# ----------------------------------------------------------------------------
def build_bass():
    import concourse.bass as bass
    import concourse.tile as tile
    from concourse import bacc, mybir

    f16 = mybir.dt.float16
    f32 = mybir.dt.float32
    AF = mybir.ActivationFunctionType
    OP = mybir.AluOpType
    AX = mybir.AxisListType

    nc = bacc.Bacc("TRN2", target_bir_lowering=False, debug=False, num_devices=NC_)
    RG = [list(range(NC_))]

    def din(name, shape, dtype):
        return nc.dram_tensor(name, list(shape), dtype, kind="ExternalInput").ap()

    d = {}
    d["lt"] = din("lt", [P, 8, P], f16)
    d["latlon32"] = din("latlon32", [P, 2], f32)
    d["AT"] = din("AT", [AUG, H, AUG], f16)
    d["Wv0az"] = din("Wv0az", [AUG, H, D + 1], f16)
    d["Wr0a"] = din("Wr0a", [AUG, DM], f16)
    d["kxTaug_all"] = din("kxTaug_all", [K, AUG, N], f16)
    d["kxaug_all"] = din("kxaug_all", [K, P, 8, AUG], f16)
    d["kxTaug_loc"] = din("kxTaug_loc", [K, AUG, P], f16)
    d["Wq1s"] = din("Wq1s", [P, 2, H * D], f16)
    d["Wk1"] = din("Wk1", [P, 2, H * D], f16)
    d["Wv1"] = din("Wv1", [P, 2, H * D], f16)
    d["bq1b"] = din("bq1b", [P, 8, P], f32)
    d["bk1b"] = din("bk1b", [P, 8, P], f32)
    d["bv1"] = din("bv1", [1, H * D], f16)
    d["Wr1"] = din("Wr1", [P, 2, DM], f16)
    d["brpe1"] = din("brpe1", [NG, 1, DM], f16)
    d["Wqe"] = din("Wqe", [P, 2, DM], f16)
    d["Wke"] = din("Wke", [P, 2, DM], f16)
    d["Wve"] = din("Wve", [P, 2, DM], f16)
    d["Wo"] = din("Wo", [P, 2, DM], f16)
    d["bqe"] = din("bqe", [1, DM], f16)
    d["bke"] = din("bke", [1, DM], f16)
    d["bve"] = din("bve", [1, DM], f16)
    d["bo"] = din("bo", [1, DM], f16)
    d["W1"] = din("W1", [P, 2, FF], f16)
    d["b1f"] = din("b1f", [1, FF], f16)
    d["W2"] = din("W2", [P, 16, DM], f16)
    d["b2f"] = din("b2f", [1, DM], f16)
    d["Wd"] = din("Wd", [P, 2, F - 2], f16)
    d["bd"] = din("bd", [1, F - 2], f16)
    d["g1"] = din("g1", [1, DM], f32)
    d["be1"] = din("be1", [1, DM], f32)
    d["g2"] = din("g2", [1, DM], f32)
    d["be2"] = din("be2", [1, DM], f32)
    d["xTaug_init"] = din("xTaug_init", [AUG, P], f16)
    d["xaug_init"] = din("xaug_init", [P, AUG], f16)
    d["idn"] = din("idn", [P, P], f16)
    d["ones1"] = din("ones1", [1, P], f16)

    out_d = nc.dram_tensor("out", [N_GEN, P, F], f32, kind="ExternalOutput").ap()

    mm = nc.tensor.matmul
    V = nc.vector
    S = nc.scalar
    G = nc.gpsimd

    def bc(ap, ins_size):
        """Insert a stride-0 dim before the last free dim: [p, n] -> [p, m, n]."""
        return bass.AP(tensor=ap.tensor, offset=ap.offset,
                       ap=[list(ap.ap[0]), [0, ins_size], list(ap.ap[1])])

    from contextlib import ExitStack
    with tile.TileContext(nc) as tc, ExitStack() as ctx:
        consts = ctx.enter_context(tc.tile_pool(name="consts", bufs=1))
        state = ctx.enter_context(tc.tile_pool(name="state", bufs=1))
        work = ctx.enter_context(tc.tile_pool(name="work", bufs=2))
        big = ctx.enter_context(tc.tile_pool(name="big", bufs=1))
        psA = ctx.enter_context(tc.tile_pool(name="psA", bufs=2, space="PSUM"))
        psS = ctx.enter_context(tc.tile_pool(name="psS", bufs=2, space="PSUM"))
        psM = ctx.enter_context(tc.tile_pool(name="psM", bufs=2, space="PSUM"))
        dram = ctx.enter_context(tc.tile_pool(name="dram", bufs=2, space="DRAM"))

        def ps_big(name, shape=(P, 8, P)):
            return psA.tile(list(shape), f32, name=name, tag="SP")

        def ps_m(shape, name, dtype=None):
            return psM.tile(list(shape), dtype or f32, name=name, tag="mP")

        cs = {}
        for name, ap in d.items():
            if name in ("g1", "be1", "g2", "be2", "kxTaug_all", "kxaug_all",
                        "kxTaug_loc", "xTaug_init", "xaug_init", "latlon32"):
                continue
            if name == "brpe1":
                continue
            t = consts.tile(list(ap.shape), ap.dtype, name=f"c_{name}")
            nc.sync.dma_start(out=t, in_=ap)
            cs[name] = t
        for name in ("g1", "be1", "g2", "be2"):
            t = consts.tile([P, DM], f32, name=f"c_{name}")
            nc.sync.dma_start(out=t, in_=d[name].to_broadcast((P, DM)))
            cs[name] = t
        latlon32 = consts.tile([P, 2], f32, name="latlon32")
        nc.sync.dma_start(out=latlon32, in_=d["latlon32"])

        idn = cs["idn"]; ones1 = cs["ones1"]
        lt = cs["lt"]
        eps1 = consts.tile([P, 1], f32, name="eps1")
        V.memset(eps1, 1e-5)

        Kc = state.tile([P, TCACHE, DM], f16, name="Kc")
        Vc2 = state.tile([P, H, DH, TCACHE], f16, name="Vc2")
        xTaug = state.tile([AUG, P], f16, name="xTaug")
        xaug = state.tile([P, AUG], f16, name="xaug")
        nc.sync.dma_start(out=xTaug, in_=d["xTaug_init"])
        nc.sync.dma_start(out=xaug, in_=d["xaug_init"])

        # ------------------------------------------------------------------
        def transpose128(src_ap, n_chunks, name):
            dst = work.tile([P, n_chunks, P], f16, name=name,
                            bufs=(4 if name == "midT" else None))
            for fc in range(n_chunks):
                pt = ps_m([P, P], "ptp", dtype=f16)
                mm(pt, src_ap[:, fc * P:(fc + 1) * P], idn, start=True, stop=True,
                   is_transpose=True)
                V.tensor_copy(out=dst[:, fc, :], in_=pt)
            return dst

        def combine(agg01, agg23, rootP, name):
            """sum_h agg_h/(4 z_h) + rootP; z_h rides in agg col 256."""
            zi = work.tile([P, H], f32, name=f"zi_{name}")
            V.tensor_scalar(out=zi[:, 0:2],
                            in0=agg01[:, :, 256:257].rearrange("p h x -> p (h x)"),
                            scalar1=4.0, scalar2=4e-16, op0=OP.mult, op1=OP.add)
            V.tensor_scalar(out=zi[:, 2:4],
                            in0=agg23[:, :, 256:257].rearrange("p h x -> p (h x)"),
                            scalar1=4.0, scalar2=4e-16, op0=OP.mult, op1=OP.add)
            V.reciprocal(out=zi, in_=zi)
            t0 = work.tile([P, DM], f32, name=f"cmb_{name}")
            V.tensor_scalar_mul(out=t0, in0=agg01[:, 0, 0:256], scalar1=zi[:, 0:1])
            V.scalar_tensor_tensor(out=t0, in0=agg01[:, 1, 0:256],
                                   scalar=zi[:, 1:2], in1=t0,
                                   op0=OP.mult, op1=OP.add)
            V.scalar_tensor_tensor(out=t0, in0=agg23[:, 0, 0:256],
                                   scalar=zi[:, 2:3], in1=t0,
                                   op0=OP.mult, op1=OP.add)
            V.scalar_tensor_tensor(out=t0, in0=agg23[:, 1, 0:256],
                                   scalar=zi[:, 3:4], in1=t0,
                                   op0=OP.mult, op1=OP.add)
            out_nm = work.tile([P, DM], f16, name=f"nm_{name}")
            V.tensor_tensor(out=out_nm, in0=rootP, in1=t0, op=OP.add)
            return out_nm

        def tconv0(xTa_all_fn, xa_all_fn, xTa_loc):
            """Rank-11 layer-0 tconv."""
            MpP = ps_m([AUG, H, P], "MpP")
            for h in range(H):
                mm(MpP[:, h, :], cs["AT"][:, h, :], xTa_loc, start=True, stop=True)
            Mp = work.tile([AUG, H, P], f16, name="Mp")
            V.tensor_copy(out=Mp, in_=MpP)
            PT = big.tile([P, H, 8, P], f16, name="PT", tag="PT", bufs=2)
            GTP = ps_m([AUG, H, P], "GTP")
            for cc in range(8):
                sp = psS.tile([P, H, P], f32, name="Scc", tag="Scc")
                mm(sp, idn, bc(lt[:, cc, :], H), start=True, stop=False)
                for h in range(H):
                    mm(sp[:, h, :], xTa_all_fn(cc), Mp[:, h, :],
                       start=False, stop=True)
                S.activation(out=PT[:, :, cc, :], in_=sp, func=AF.Exp)
            for cc in range(8):
                mm(GTP, xa_all_fn(cc), PT[:, :, cc, :],
                   start=(cc == 0), stop=(cc == 7))
            GT = work.tile([AUG, H, P], f16, name="GT")
            V.tensor_copy(out=GT, in_=GTP)
            agg01 = ps_big("agg01", (P, 2, 512))
            agg23 = ps_big("agg23", (P, 2, 512))
            for h in range(H):
                dst = (agg01, agg23)[h // 2][:, h % 2, 0:D + 1]
                mm(dst, GT[:, h, :], cs["Wv0az"][:, h, :], start=True, stop=True)
            rootP = ps_m([P, DM], "rootP")
            mm(rootP, xTa_loc, cs["Wr0a"], start=True, stop=True)
            mid = combine(agg01, agg23, rootP, "t0")
            return transpose128(mid, 2, "midT")

        def proj_T(midT, W, b, name):
            dst_sb = work.tile([P, 8, P], f16, name=name)
            pp = ps_big("SP")
            for cc in range(8):
                for fc in range(2):
                    mm(pp[:, cc, :], W[:, fc, cc * P:(cc + 1) * P],
                       midT[:, fc, :], start=(fc == 0), stop=(fc == 1))
            V.tensor_tensor(out=dst_sb, in0=pp, in1=b, op=OP.add)
            return dst_sb

        def tconv1_proj(midT):
            """Local K/V projections: k1T [128,8,128] f16, v [128,1024] f16."""
            kT = proj_T(midT, cs["Wk1"], cs["bk1b"], "k1T")
            vloc = work.tile([P, H * D], f16, name="v1loc")
            for hf in range(2):
                vp = ps_m([P, 512], "vP")
                mm(vp, ones1, cs["bv1"][0:1, hf * 512:(hf + 1) * 512],
                   start=True, stop=False)
                for fc in range(2):
                    mm(vp, midT[:, fc, :], cs["Wv1"][:, fc, hf * 512:(hf + 1) * 512],
                       start=False, stop=(fc == 1))
                V.tensor_copy(out=vloc[:, hf * 512:(hf + 1) * 512], in_=vp)
            return kT, vloc

        def tconv1_attn(midT, kst, vall, idx, brpe):
            """Dense edge attention from gathered K/V. Returns (h_nm, hT)."""
            qT = proj_T(midT, cs["Wq1s"], cs["bq1b"], "q1T")
            PT = big.tile([P, H, 8, P], f16, name="PT", tag="PT", bufs=2)
            for cc in range(8):
                sp = psS.tile([P, H, P], f32, name="Scc", tag="Scc")
                mm(sp, idn, bc(lt[:, cc, :], H), start=True, stop=False)
                for h in range(H):
                    for dc in range(2):
                        mm(sp[:, h, :], kst[:, cc, 2 * h + dc, :],
                           qT[:, 2 * h + dc, :], start=False, stop=(dc == 1))
                S.activation(out=PT[:, :, cc, :], in_=sp, func=AF.Exp)
            agg01 = ps_big("agg01", (P, 2, 512))
            agg23 = ps_big("agg23", (P, 2, 512))
            for h in range(H):
                dst = (agg01, agg23)[h // 2][:, h % 2, 0:D + 1]
                for cc in range(8):
                    mm(dst, PT[:, h, cc, :], vall[:, cc, h, 0:D + 1],
                       start=(cc == 0), stop=(cc == 7))
            rootP = ps_m([P, DM], "rootP")
            mm(rootP, ones1, brpe, start=True, stop=False)
            for fc in range(2):
                mm(rootP, midT[:, fc, :], cs["Wr1"][:, fc, :],
                   start=False, stop=(fc == 1))
            h_nm = combine(agg01, agg23, rootP, "t1")
            hT = transpose128(h_nm, 2, "hT")
            return h_nm, hT

        def cache_update(slot, hT):
            for W, b, which in ((cs["Wke"], cs["bke"], "k"),
                                (cs["Wve"], cs["bve"], "v")):
                pp = ps_m([P, DM], "cuP")
                mm(pp, ones1, b, start=True, stop=False)
                for fc in range(2):
                    mm(pp, hT[:, fc, :], W[:, fc, :], start=False, stop=(fc == 1))
                if which == "k":
                    V.tensor_copy(out=Kc[:, slot, :], in_=pp)
                else:
                    V.tensor_copy(out=Vc2[:, :, :, slot],
                                  in_=pp.rearrange("p (h e) -> p h e", h=H))

        def layer_norm(x_f32_psum, resid_f16, g, be, name):
            t1 = work.tile([P, DM], f32, name=f"ln_t1_{name}")
            V.tensor_tensor(out=t1, in0=x_f32_psum, in1=resid_f16, op=OP.add)
            st = work.tile([P, 6], f32, name=f"ln_st_{name}")
            V.bn_stats(out=st, in_=t1)
            mv = work.tile([P, 2], f32, name=f"ln_mv_{name}")
            V.bn_aggr(out=mv, in_=st)
            rs = work.tile([P, 1], f32, name=f"ln_rs_{name}")
            S.activation(out=rs, in_=mv[:, 1:2], func=AF.Ln, bias=eps1[:, 0:1])
            S.activation(out=rs, in_=rs, func=AF.Exp, scale=-0.5)
            V.tensor_scalar(out=t1, in0=t1, scalar1=mv[:, 0:1], scalar2=rs,
                            op0=OP.subtract, op1=OP.mult)
            V.tensor_tensor(out=t1, in0=t1, in1=g, op=OP.mult)
            o = work.tile([P, DM], f16, name=f"ln_o_{name}")
            V.tensor_tensor(out=o, in0=t1, in1=be, op=OP.add)
            return o

        def enc(t, hT_last, h_nm_last):
            qp = ps_m([P, DM], "qeP")
            mm(qp, ones1, cs["bqe"], start=True, stop=False)
            for fc in range(2):
                mm(qp, hT_last[:, fc, :], cs["Wqe"][:, fc, :],
                   start=False, stop=(fc == 1))
            q = work.tile([P, DM], f16, name="qe")
            V.tensor_copy(out=q, in_=qp)
            sc = work.tile([P, H, TCACHE], f32, name="sc")
            tmp = work.tile([P, TCACHE, DH], f32, name="sctmp", tag="etmp")
            for h in range(H):
                V.tensor_tensor(out=tmp[:, 0:t, :],
                                in0=Kc[:, 0:t, h * DH:(h + 1) * DH],
                                in1=bc(q[:, h * DH:(h + 1) * DH], t),
                                op=OP.mult)
                V.tensor_reduce(out=sc[:, h, 0:t], in_=tmp[:, 0:t, :], axis=AX.X,
                                op=OP.add)
            S.activation(out=sc[:, :, 0:t], in_=sc[:, :, 0:t], func=AF.Exp)
            z = work.tile([P, H], f32, name="ze")
            V.tensor_reduce(out=z, in_=sc[:, :, 0:t], axis=AX.X, op=OP.add)
            V.reciprocal(out=z, in_=z)
            o = work.tile([P, DM], f16, name="oe")
            tmp2 = work.tile([P, DH, TCACHE], f32, name="otmp", tag="etmp")
            orow = work.tile([P, DH], f32, name="orow")
            for h in range(H):
                V.tensor_tensor(out=tmp2[:, :, 0:t], in0=Vc2[:, h, :, 0:t],
                                in1=bc(sc[:, h, 0:t], DH), op=OP.mult)
                V.tensor_reduce(out=orow, in_=tmp2[:, :, 0:t], axis=AX.X,
                                op=OP.add)
                V.tensor_scalar_mul(out=o[:, h * DH:(h + 1) * DH], in0=orow,
                                    scalar1=z[:, h:h + 1])
            oT = transpose128(o, 2, "oT")
            aop = ps_m([P, DM], "aoP")
            mm(aop, ones1, cs["bo"], start=True, stop=False)
            for fc in range(2):
                mm(aop, oT[:, fc, :], cs["Wo"][:, fc, :], start=False,
                   stop=(fc == 1))
            h1 = layer_norm(aop, h_nm_last, cs["g1"], cs["be1"], "1")
            h1T = transpose128(h1, 2, "h1T")
            zT = work.tile([P, 16, P], f16, name="zT", bufs=1)
            for half in range(2):
                zp = ps_big("SP")
                for s8 in range(8):
                    ffc = half * 8 + s8
                    mm(zp[:, s8, :], cs["b1f"][0:1, ffc * P:(ffc + 1) * P], ones1,
                       start=True, stop=False)
                    for fc in range(2):
                        mm(zp[:, s8, :], cs["W1"][:, fc, ffc * P:(ffc + 1) * P],
                           h1T[:, fc, :], start=False, stop=(fc == 1))
                S.activation(out=zT[:, half * 8:(half + 1) * 8, :], in_=zp,
                             func=AF.Relu)
            y2p = ps_m([P, DM], "y2P")
            mm(y2p, ones1, cs["b2f"], start=True, stop=False)
            for ffc in range(16):
                mm(y2p, zT[:, ffc, :], cs["W2"][:, ffc, :],
                   start=False, stop=(ffc == 15))
            ctx_ = layer_norm(y2p, h1, cs["g2"], cs["be2"], "2")
            ctxT = transpose128(ctx_, 2, "ctxT")
            return ctx_, ctxT

        # ------------------------------------------------------------------
        # Phase K: known steps with ONE batched AllGather
        # ------------------------------------------------------------------
        def tconv1_stage(midT, idx):
            kT, vloc = tconv1_proj(midT)
            binkv = dram.tile([2 * P, H * D], f16, name="binkv")
            boutkv = dram.tile([NC_ * 2 * P, H * D], f16, name="boutkv",
                               addr_space="Shared")
            nc.sync.dma_start(
                out=binkv[0:P, :].rearrange("p (c j) -> p c j", j=P), in_=kT)
            nc.sync.dma_start(out=binkv[P:2 * P, :], in_=vloc)
            nc.gpsimd.collective_compute(
                "AllGather", OP.bypass, replica_groups=RG,
                ins=[binkv[:]], outs=[boutkv[:]])
            gkv = boutkv.rearrange("(r t p) hd -> r t p hd", r=NC_, t=2)
            kst = big.tile([P, 8, 8, P], f16, name="kst", tag="kst", bufs=1)
            vall = big.tile([P, 8, H, D + 4], f16, name="vall", tag="vall", bufs=1)
            for r in range(NC_):
                ke = G if r < 4 else nc.scalar
                ve = nc.scalar if r < 4 else G
                ke.dma_start(out=kst[:, r, :, :],
                             in_=gkv[r, 0].rearrange("p (c j) -> p c j", j=P))
                ve.dma_start(out=vall[:, r, :, 0:D],
                             in_=gkv[r, 1].rearrange("p (h e) -> p h e", h=H))
            V.memset(vall[:, :, :, D:D + 1], 1.0)
            brpe = work.tile([1, DM], f16, name="brpe")
            nc.sync.dma_start(out=brpe, in_=d["brpe1"][idx])
            return (midT, kst, vall, brpe, idx)

        def tconv1_finish(midT, kst, vall, brpe, idx):
            return tconv1_attn(midT, kst, vall, idx, brpe)

        def tconv1_gather_attn(midT, idx):
            return tconv1_finish(*tconv1_stage(midT, idx))

        hT_last = None
        h_nm_last = None
        pending = None
        for i in range(N_KNOWN):
            kxTa = work.tile([AUG, N], f16, name="kxTa", bufs=3)
            nc.sync.dma_start(out=kxTa, in_=d["kxTaug_all"][i])
            kxa = work.tile([P, 8, AUG], f16, name="kxa", bufs=3)
            nc.sync.dma_start(out=kxa, in_=d["kxaug_all"][i])
            kxTl = work.tile([AUG, P], f16, name="kxTl", bufs=3)
            nc.sync.dma_start(out=kxTl, in_=d["kxTaug_loc"][i])
            midT = tconv0(lambda cc, _t=kxTa: _t[:, cc * P:(cc + 1) * P],
                          lambda cc, _t=kxa: _t[:, cc, :], kxTl)
            staged = tconv1_stage(midT, i)
            if pending is not None:
                h_nm_last, hT_last = tconv1_finish(*pending)
                cache_update(pending[-1], hT_last)
            pending = staged
        h_nm_last, hT_last = tconv1_finish(*pending)
        cache_update(pending[-1], hT_last)

        # ------------------------------------------------------------------
        # Phase G: autoregressive generation
        # ------------------------------------------------------------------
        for t in range(K, K + N_GEN):
            ctx_, ctxT = enc(t, hT_last, h_nm_last)
            wdp = ps_m([P, F - 2], "wdP")
            mm(wdp, ones1, cs["bd"], start=True, stop=False)
            for fc in range(2):
                mm(wdp, ctxT[:, fc, :], cs["Wd"][:, fc, :],
                   start=False, stop=(fc == 1))
            xn32 = work.tile([P, F - 2], f32, name="xn32")
            V.tensor_copy(out=xn32, in_=wdp)
            nc.sync.dma_start(out=out_d[t - K, :, 0:2], in_=d["latlon32"])
            nc.sync.dma_start(out=out_d[t - K, :, 2:F], in_=xn32)
            if t == K + N_GEN - 1 or t == NG - 1:
                break
            V.tensor_copy(out=xaug[:, 0:F - 2], in_=wdp)
            tp = ps_m([F - 2, P], "ptp", dtype=f16)
            mm(tp, xaug[:, 0:F - 2], idn, start=True, stop=True, is_transpose=True)
            V.tensor_copy(out=xTaug[0:F - 2, :], in_=tp)
            gin = dram.tile([1, 2 * AUG * P], f16, name="g_in")
            gout = dram.tile([NC_, 2 * AUG * P], f16, name="g_out",
                             addr_space="Shared")
            G.dma_start(
                out=gin[0, 0:AUG * P].rearrange("(p j) -> p j", p=AUG), in_=xTaug)
            G.dma_start(
                out=gin[0, AUG * P:2 * AUG * P].rearrange("(p j) -> p j", p=P),
                in_=xaug)
            nc.gpsimd.collective_compute(
                "AllGather", OP.bypass, replica_groups=RG,
                ins=[gin[:]], outs=[gout[:]])
            xTa_all = work.tile([AUG, 8, P], f16, name="xTa_all")
            G.dma_start(
                out=xTa_all,
                in_=gout[:, 0:AUG * P].rearrange("r (p j) -> p r j", p=AUG))
            xa_all = work.tile([P, 8, AUG], f16, name="xa_all")
            G.dma_start(
                out=xa_all,
                in_=gout[:, AUG * P:2 * AUG * P].rearrange("r (p j) -> p r j", p=P))
            midT = tconv0(lambda cc, _t=xTa_all: _t[:, cc, :],
                          lambda cc, _t=xa_all: _t[:, cc, :], xTaug)
            h_nm_last, hT_last = tconv1_gather_attn(midT, t)
            cache_update(t, hT_last)

    nc.finalize()
    return nc


# ----------------------------------------------------------------------------
# Host-side preprocessing
# ----------------------------------------------------------------------------
def prep_in_maps(inputs):
    f32 = np.float32
    f16 = np.float16
    g = {k: np.asarray(v) for k, v in inputs.items()}
    kx = g["known_x"].astype(f32)                       # [10, 1024, 10]
    ei = g["edge_index"].astype(np.int64)

    Cnt = np.zeros((N, N), f32)
    np.add.at(Cnt, (ei[0], ei[1]), 1.0)
    LT = np.where(Cnt > 0, np.log(np.maximum(Cnt, 1.0)), _NEG).astype(f32)

    isd = f32(1.0 / np.sqrt(D))
    PERM = [2, 3, 4, 5, 6, 7, 8, 9, 0, 1, 10]
    Wq0a = (np.vstack([g["Wq0"], g["bq0"][None]]).astype(f32) * isd)[PERM]
    Wk0a = np.vstack([g["Wk0"], g["bk0"][None]]).astype(f32)[PERM]
    A = np.stack([(Wk0a[:, h * D:(h + 1) * D] @ Wq0a[:, h * D:(h + 1) * D].T)
                  for h in range(H)])                                # [4, 11, 11]
    AT = A.transpose(0, 2, 1).transpose(1, 0, 2).copy()              # [11, 4, 11]

    kxaug = np.concatenate([kx, np.ones((K, N, 1), f32)], axis=2)[:, :, PERM]
    kxTaug = kxaug.transpose(0, 2, 1).copy()                         # [10, 11, 1024]

    ide = f32(1.0 / np.sqrt(DH))
    Wqkv, bqkv = g["Wqkv"].astype(f32), g["bqkv"].astype(f32)

    def w2t(w, nch):
        m = w.shape[1]
        return np.ascontiguousarray(
            np.asarray(w, f32).reshape(nch, P, m).transpose(1, 0, 2))

    Wv0a = np.vstack([g["Wv0"], g["bv0"][None]]).astype(f32)[PERM]   # [11, 1024]
    Wv0az = np.zeros((AUG, H, D + 1), f32)
    for h in range(H):
        Wv0az[:, h, 0:D] = Wv0a[:, h * D:(h + 1) * D]
    Wv0az[10, :, D] = 1.0      # e10 column (ones-row index under PERM) -> z

    common = {
        "AT": AT.astype(f16),
        "Wv0az": Wv0az.astype(f16),
        "Wr0a": np.vstack([g["Wr0"], g["br0"][None]])[PERM].astype(f16),
        "Wq1s": w2t(np.asarray(g["Wq1"], f32) * isd, 2).astype(f16),
        "Wk1": w2t(g["Wk1"], 2).astype(f16),
        "Wv1": w2t(g["Wv1"], 2).astype(f16),
        "bq1b": np.ascontiguousarray(np.broadcast_to(
            (np.asarray(g["bq1"], f32) * isd).reshape(8, P).T[:, :, None],
            (P, 8, P))).astype(f32),
        "bk1b": np.ascontiguousarray(np.broadcast_to(
            np.asarray(g["bk1"], f32).reshape(8, P).T[:, :, None],
            (P, 8, P))).astype(f32),
        "bv1": np.asarray(g["bv1"], f16)[None],
        "Wr1": w2t(g["Wr1"], 2).astype(f16),
        "brpe1": (np.asarray(g["br1"], f32)[None]
                  + np.asarray(g["pe"], f32))[:, None, :].astype(f16),
        "Wqe": w2t(Wqkv[:, 0:DM] * ide, 2).astype(f16),
        "Wke": w2t(Wqkv[:, DM:2 * DM], 2).astype(f16),
        "Wve": w2t(Wqkv[:, 2 * DM:], 2).astype(f16),
        "bqe": (bqkv[0:DM] * ide)[None].astype(f16),
        "bke": bqkv[DM:2 * DM][None].astype(f16),
        "bve": bqkv[2 * DM:][None].astype(f16),
        "Wo": w2t(g["Wo"], 2).astype(f16),
        "bo": np.asarray(g["bo"], f16)[None],
        "W1": w2t(g["W1"], 2).astype(f16),
        "b1f": np.asarray(g["b1f"], f16)[None],
        "W2": w2t(g["W2"], 16).astype(f16),
        "b2f": np.asarray(g["b2f"], f16)[None],
        "Wd": w2t(g["Wd"], 2).astype(f16),
        "bd": np.asarray(g["bd"], f16)[None],
        "g1": np.asarray(g["g1"], f32)[None],
        "be1": np.asarray(g["be1"], f32)[None],
        "g2": np.asarray(g["g2"], f32)[None],
        "be2": np.asarray(g["be2"], f32)[None],
        "idn": np.eye(P, dtype=f16),
        "ones1": np.ones((1, P), f16),
        "kxTaug_all": kxTaug.astype(f16),
        "kxaug_all": np.ascontiguousarray(
            kxaug.reshape(K, 8, P, AUG).transpose(0, 2, 1, 3)).astype(f16),
    }
    in_maps = []
    for c in range(NC_):
        sl = slice(P * c, P * (c + 1))
        m = dict(common)
        m["lt"] = np.ascontiguousarray(
            LT[:, sl].reshape(8, P, P).transpose(1, 0, 2)).astype(f16)
        m["latlon32"] = np.ascontiguousarray(kx[K - 1, sl, 0:2]).astype(f32)
        ll = kx[K - 1, sl, 0:2].astype(f32)
        xti = np.zeros((AUG, P), f32); xti[8:10] = ll.T; xti[10] = 1.0
        m["xTaug_init"] = xti.astype(f16)
        xai = np.zeros((P, AUG), f32); xai[:, 8:10] = ll; xai[:, 10] = 1.0
        m["xaug_init"] = xai.astype(f16)
        m["kxTaug_loc"] = np.ascontiguousarray(kxTaug[:, :, sl]).astype(f16)
        in_maps.append(m)
    return in_maps


_CACHED = {}


def run(inputs, trace=False, trace_kwargs=None):
    from concourse import bass_utils
    if "nc" not in _CACHED:
        _CACHED["nc"] = build_bass()
    in_maps = prep_in_maps(inputs)
    res = bass_utils.run_bass_kernel_spmd(
        _CACHED["nc"], in_maps, core_ids=list(range(NC_)), trace=trace,
        **(trace_kwargs or {}))
    out = np.concatenate([res.results[c]["out"] for c in range(NC_)], axis=1)
    return out.astype(np.float32), res


def kernel(**inputs):
    out, _ = run(inputs, trace=False)
    return out


# revision 19
# speedup vs baseline: 1.0152x; 1.0152x over previous
"""Trainium2 Bass kernel for nn_AdaptiveConditionedGraphTransformer.

Strategy (8 NeuronCores, data-parallel over nodes, 128 nodes/core):
- Graph edge-attention (PyG TransformerConv) computed DENSELY per core as
  [src=1024, dst=128] score matrices on TensorE; duplicate edges + masking
  handled exactly via a host-precomputed ln(count) additive mask injected
  into PSUM with an identity matmul inside the QK^T accumulation group.
- Layer-0 tconv (fin=10) uses a rank-11 factorization: S = x_aug A x_aug^T
  with A = Wk_aug Wq_aug^T/16 precomputed on host -> no K/V materialization
  and only a 5.5KB/rank AllGather of x_aug per autoregressive step.
- Layer-1 tconv computes K^T/V locally; the 10 independent known steps share
  ONE batched 5MB AllGather; autoregressive steps use split K/V AllGathers
  so score matmuls overlap the V gather.
- Softmax denominators ride along in the aggregation matmuls (ones column
  appended to V / e10 column appended to Wv0aug) -- no separate z matmuls.
- Temporal transformer layer: exact KV-cache, last-query only, on VectorE.
- LayerNorm rsqrt via exp(-0.5*ln(var+eps)) so ScalarE stays on the single
  natural_log_exp table set the whole kernel.
- All matmul inputs fp16 (fp32 PSUM accumulate); softmax/LN math fp32.

kernel(**inputs) takes FULL inputs, shards internally, returns FULL output.
"""
import os
import sys

import numpy as np

sys.path.insert(0, "/opt/trn_rl_repo")

N, E, F = 1024, 16384, 10
DM, H = 256, 4
D = 256
NG, K = 20, 10
FF = 2048
DH = DM // H
NC_ = 8
P = N // NC_          # 128 nodes per core
AUG = F + 1           # 11
TCACHE = NG - 1       # 19 cache slots

N_KNOWN = int(os.environ.get("GT_KNOWN", "10"))
N_GEN = int(os.environ.get("GT_GEN", "10"))

_NEG = -30000.0


# ----------------------------------------------------------------------------
# Device program
# ----------------------------------------------------------------------------
def build_bass():
    import concourse.bass as bass
    import concourse.tile as tile
    from concourse import bacc, mybir

    f16 = mybir.dt.float16
    f32 = mybir.dt.float32
    AF = mybir.ActivationFunctionType
    OP = mybir.AluOpType
    AX = mybir.AxisListType

    nc = bacc.Bacc("TRN2", target_bir_lowering=False, debug=False, num_devices=NC_)
    RG = [list(range(NC_))]

    def din(name, shape, dtype):
        return nc.dram_tensor(name, list(shape), dtype, kind="ExternalInput").ap()

    d = {}
    d["lt"] = din("lt", [P, 8, P], f16)
    d["latlon32"] = din("latlon32", [P, 2], f32)
    d["AT"] = din("AT", [AUG, H, AUG], f16)
    d["Wv0az"] = din("Wv0az", [AUG, H, D + 1], f16)
    d["Wr0a"] = din("Wr0a", [AUG, DM], f16)
    d["kxTaug_all"] = din("kxTaug_all", [K, AUG, N], f16)
    d["kxaug_all"] = din("kxaug_all", [K, P, 8, AUG], f16)
    d["kxTaug_loc"] = din("kxTaug_loc", [K, AUG, P], f16)
    d["Wq1s"] = din("Wq1s", [P, 2, H * D], f16)
    d["Wk1"] = din("Wk1", [P, 2, H * D], f16)
    d["Wv1"] = din("Wv1", [P, 2, H * D], f16)
    d["bq1b"] = din("bq1b", [P, 8, P], f32)
    d["bk1b"] = din("bk1b", [P, 8, P], f32)
    d["bv1"] = din("bv1", [1, H * D], f16)
    d["Wr1"] = din("Wr1", [P, 2, DM], f16)
    d["brpe1"] = din("brpe1", [NG, 1, DM], f16)
    d["Wqe"] = din("Wqe", [P, 2, DM], f16)
    d["Wke"] = din("Wke", [P, 2, DM], f16)
    d["Wve"] = din("Wve", [P, 2, DM], f16)
    d["Wo"] = din("Wo", [P, 2, DM], f16)
    d["bqe"] = din("bqe", [1, DM], f16)
    d["bke"] = din("bke", [1, DM], f16)
    d["bve"] = din("bve", [1, DM], f16)
    d["bo"] = din("bo", [1, DM], f16)
    d["W1"] = din("W1", [P, 2, FF], f16)
    d["b1f"] = din("b1f", [1, FF], f16)
    d["W2"] = din("W2", [P, 16, DM], f16)
    d["b2f"] = din("b2f", [1, DM], f16)
    d["Wd"] = din("Wd", [P, 2, F - 2], f16)
    d["bd"] = din("bd", [1, F - 2], f16)
    d["g1"] = din("g1", [1, DM], f32)
    d["be1"] = din("be1", [1, DM], f32)
    d["g2"] = din("g2", [1, DM], f32)
    d["be2"] = din("be2", [1, DM], f32)
    d["xTaug_init"] = din("xTaug_init", [AUG, P], f16)
    d["xaug_init"] = din("xaug_init", [P, AUG], f16)
    d["idn"] = din("idn", [P, P], f16)
    d["ones1"] = din("ones1", [1, P], f16)

    out_d = nc.dram_tensor("out", [N_GEN, P, F], f32, kind="ExternalOutput").ap()

    mm = nc.tensor.matmul
    V = nc.vector
    S = nc.scalar
    G = nc.gpsimd

    def bc(ap, ins_size):
        """Insert a stride-0 dim before the last free dim: [p, n] -> [p, m, n]."""
        return bass.AP(tensor=ap.tensor, offset=ap.offset,
                       ap=[list(ap.ap[0]), [0, ins_size], list(ap.ap[1])])

    from contextlib import ExitStack
    with tile.TileContext(nc) as tc, ExitStack() as ctx:
        consts = ctx.enter_context(tc.tile_pool(name="consts", bufs=1))
        state = ctx.enter_context(tc.tile_pool(name="state", bufs=1))
        work = ctx.enter_context(tc.tile_pool(name="work", bufs=2))
        big = ctx.enter_context(tc.tile_pool(name="big", bufs=1))
        psA = ctx.enter_context(tc.tile_pool(name="psA", bufs=2, space="PSUM"))
        psS = ctx.enter_context(tc.tile_pool(name="psS", bufs=2, space="PSUM"))
        psM = ctx.enter_context(tc.tile_pool(name="psM", bufs=2, space="PSUM"))
        dram = ctx.enter_context(tc.tile_pool(name="dram", bufs=2, space="DRAM"))

        def ps_big(name, shape=(P, 8, P)):
            return psA.tile(list(shape), f32, name=name, tag="SP")

        def ps_m(shape, name, dtype=None):
            return psM.tile(list(shape), dtype or f32, name=name, tag="mP")

        cs = {}
        for name, ap in d.items():
            if name in ("g1", "be1", "g2", "be2", "kxTaug_all", "kxaug_all",
                        "kxTaug_loc", "xTaug_init", "xaug_init", "latlon32"):
                continue
            if name == "brpe1":
                continue
            t = consts.tile(list(ap.shape), ap.dtype, name=f"c_{name}")
            nc.sync.dma_start(out=t, in_=ap)
            cs[name] = t
        for name in ("g1", "be1", "g2", "be2"):
            t = consts.tile([P, DM], f32, name=f"c_{name}")
            nc.sync.dma_start(out=t, in_=d[name].to_broadcast((P, DM)))
            cs[name] = t
        latlon32 = consts.tile([P, 2], f32, name="latlon32")
        nc.sync.dma_start(out=latlon32, in_=d["latlon32"])

        idn = cs["idn"]; ones1 = cs["ones1"]
        lt = cs["lt"]
        eps1 = consts.tile([P, 1], f32, name="eps1")
        V.memset(eps1, 1e-5)

        Kc = state.tile([P, TCACHE, DM], f16, name="Kc")
        Vc2 = state.tile([P, H, DH, TCACHE], f16, name="Vc2")
        xTaug = state.tile([AUG, P], f16, name="xTaug")
        xaug = state.tile([P, AUG], f16, name="xaug")
        nc.sync.dma_start(out=xTaug, in_=d["xTaug_init"])
        nc.sync.dma_start(out=xaug, in_=d["xaug_init"])

        # ------------------------------------------------------------------
        def transpose128(src_ap, n_chunks, name):
            dst = work.tile([P, n_chunks, P], f16, name=name,
                            bufs=(4 if name == "midT" else None))
            for fc in range(n_chunks):
                pt = ps_m([P, P], "ptp", dtype=f16)
                mm(pt, src_ap[:, fc * P:(fc + 1) * P], idn, start=True, stop=True,
                   is_transpose=True)
                V.tensor_copy(out=dst[:, fc, :], in_=pt)
            return dst

        def combine(agg01, agg23, rootP, name):
            """sum_h agg_h/(4 z_h) + rootP; z_h rides in agg col 256."""
            zi = work.tile([P, H], f32, name=f"zi_{name}")
            V.tensor_scalar(out=zi[:, 0:2],
                            in0=agg01[:, :, 256:257].rearrange("p h x -> p (h x)"),
                            scalar1=4.0, scalar2=4e-16, op0=OP.mult, op1=OP.add)
            V.tensor_scalar(out=zi[:, 2:4],
                            in0=agg23[:, :, 256:257].rearrange("p h x -> p (h x)"),
                            scalar1=4.0, scalar2=4e-16, op0=OP.mult, op1=OP.add)
            V.reciprocal(out=zi, in_=zi)
            t0 = work.tile([P, DM], f32, name=f"cmb_{name}")
            V.tensor_scalar_mul(out=t0, in0=agg01[:, 0, 0:256], scalar1=zi[:, 0:1])
            V.scalar_tensor_tensor(out=t0, in0=agg01[:, 1, 0:256],
                                   scalar=zi[:, 1:2], in1=t0,
                                   op0=OP.mult, op1=OP.add)
            V.scalar_tensor_tensor(out=t0, in0=agg23[:, 0, 0:256],
                                   scalar=zi[:, 2:3], in1=t0,
                                   op0=OP.mult, op1=OP.add)
            V.scalar_tensor_tensor(out=t0, in0=agg23[:, 1, 0:256],
                                   scalar=zi[:, 3:4], in1=t0,
                                   op0=OP.mult, op1=OP.add)
            out_nm = work.tile([P, DM], f16, name=f"nm_{name}")
            V.tensor_tensor(out=out_nm, in0=rootP, in1=t0, op=OP.add)
            return out_nm

        def tconv0(xTa_all_fn, xa_all_fn, xTa_loc):
            """Rank-11 layer-0 tconv."""
            MpP = ps_m([AUG, H, P], "MpP")
            for h in range(H):
                mm(MpP[:, h, :], cs["AT"][:, h, :], xTa_loc, start=True, stop=True)
            Mp = work.tile([AUG, H, P], f16, name="Mp")
            V.tensor_copy(out=Mp, in_=MpP)
            PT = big.tile([P, H, 8, P], f16, name="PT", tag="PT", bufs=2)
            GTP = ps_m([AUG, H, P], "GTP")
            for cc in range(8):
                sp = psS.tile([P, H, P], f32, name="Scc", tag="Scc")
                mm(sp, idn, bc(lt[:, cc, :], H), start=True, stop=False)
                for h in range(H):
                    mm(sp[:, h, :], xTa_all_fn(cc), Mp[:, h, :],
                       start=False, stop=True)
                S.activation(out=PT[:, :, cc, :], in_=sp, func=AF.Exp)
            for cc in range(8):
                mm(GTP, xa_all_fn(cc), PT[:, :, cc, :],
                   start=(cc == 0), stop=(cc == 7))
            GT = work.tile([AUG, H, P], f16, name="GT")
            V.tensor_copy(out=GT, in_=GTP)
            agg01 = ps_big("agg01", (P, 2, 512))
            agg23 = ps_big("agg23", (P, 2, 512))
            for h in range(H):
                dst = (agg01, agg23)[h // 2][:, h % 2, 0:D + 1]
                mm(dst, GT[:, h, :], cs["Wv0az"][:, h, :], start=True, stop=True)
            rootP = ps_m([P, DM], "rootP")
            mm(rootP, xTa_loc, cs["Wr0a"], start=True, stop=True)
            mid = combine(agg01, agg23, rootP, "t0")
            return transpose128(mid, 2, "midT")

        def proj_T(midT, W, b, name):
            dst_sb = work.tile([P, 8, P], f16, name=name)
            pp = ps_big("SP")
            for cc in range(8):
                for fc in range(2):
                    mm(pp[:, cc, :], W[:, fc, cc * P:(cc + 1) * P],
                       midT[:, fc, :], start=(fc == 0), stop=(fc == 1))
            V.tensor_tensor(out=dst_sb, in0=pp, in1=b, op=OP.add)
            return dst_sb

        def tconv1_proj(midT):
            """Local K/V projections: k1T [128,8,128] f16, v [128,1024] f16."""
            kT = proj_T(midT, cs["Wk1"], cs["bk1b"], "k1T")
            vloc = work.tile([P, H * D], f16, name="v1loc")
            for hf in range(2):
                vp = ps_m([P, 512], "vP")
                mm(vp, ones1, cs["bv1"][0:1, hf * 512:(hf + 1) * 512],
                   start=True, stop=False)
                for fc in range(2):
                    mm(vp, midT[:, fc, :], cs["Wv1"][:, fc, hf * 512:(hf + 1) * 512],
                       start=False, stop=(fc == 1))
                V.tensor_copy(out=vloc[:, hf * 512:(hf + 1) * 512], in_=vp)
            return kT, vloc

        def tconv1_attn(midT, kst, vall, idx, brpe):
            """Dense edge attention from gathered K/V. Returns (h_nm, hT)."""
            qT = proj_T(midT, cs["Wq1s"], cs["bq1b"], "q1T")
            PT = big.tile([P, H, 8, P], f16, name="PT", tag="PT", bufs=2)
            for cc in range(8):
                sp = psS.tile([P, H, P], f32, name="Scc", tag="Scc")
                mm(sp, idn, bc(lt[:, cc, :], H), start=True, stop=False)
                for h in range(H):
                    for dc in range(2):
                        mm(sp[:, h, :], kst[:, cc, 2 * h + dc, :],
                           qT[:, 2 * h + dc, :], start=False, stop=(dc == 1))
                S.activation(out=PT[:, :, cc, :], in_=sp, func=AF.Exp)
            agg01 = ps_big("agg01", (P, 2, 512))
            agg23 = ps_big("agg23", (P, 2, 512))
            for h in range(H):
                dst = (agg01, agg23)[h // 2][:, h % 2, 0:D + 1]
                for cc in range(8):
                    mm(dst, PT[:, h, cc, :], vall[:, cc, h, 0:D + 1],
                       start=(cc == 0), stop=(cc == 7))
            rootP = ps_m([P, DM], "rootP")
            mm(rootP, ones1, brpe, start=True, stop=False)
            for fc in range(2):
                mm(rootP, midT[:, fc, :], cs["Wr1"][:, fc, :],
                   start=False, stop=(fc == 1))
            h_nm = combine(agg01, agg23, rootP, "t1")
            hT = transpose128(h_nm, 2, "hT")
            return h_nm, hT

        def cache_update(slot, hT):
            for W, b, which in ((cs["Wke"], cs["bke"], "k"),
                                (cs["Wve"], cs["bve"], "v")):
                pp = ps_m([P, DM], "cuP")
                mm(pp, ones1, b, start=True, stop=False)
                for fc in range(2):
                    mm(pp, hT[:, fc, :], W[:, fc, :], start=False, stop=(fc == 1))
                if which == "k":
                    V.tensor_copy(out=Kc[:, slot, :], in_=pp)
                else:
                    V.tensor_copy(out=Vc2[:, :, :, slot],
                                  in_=pp.rearrange("p (h e) -> p h e", h=H))

        def layer_norm(x_f32_psum, resid_f16, g, be, name):
            t1 = work.tile([P, DM], f32, name=f"ln_t1_{name}")
            V.tensor_tensor(out=t1, in0=x_f32_psum, in1=resid_f16, op=OP.add)
            st = work.tile([P, 6], f32, name=f"ln_st_{name}")
            V.bn_stats(out=st, in_=t1)
            mv = work.tile([P, 2], f32, name=f"ln_mv_{name}")
            V.bn_aggr(out=mv, in_=st)
            rs = work.tile([P, 1], f32, name=f"ln_rs_{name}")
            S.activation(out=rs, in_=mv[:, 1:2], func=AF.Ln, bias=eps1[:, 0:1])
            S.activation(out=rs, in_=rs, func=AF.Exp, scale=-0.5)
            V.tensor_scalar(out=t1, in0=t1, scalar1=mv[:, 0:1], scalar2=rs,
                            op0=OP.subtract, op1=OP.mult)
            V.tensor_tensor(out=t1, in0=t1, in1=g, op=OP.mult)
            o = work.tile([P, DM], f16, name=f"ln_o_{name}")
            V.tensor_tensor(out=o, in0=t1, in1=be, op=OP.add)
            return o

        def enc(t, hT_last, h_nm_last):
            qp = ps_m([P, DM], "qeP")
            mm(qp, ones1, cs["bqe"], start=True, stop=False)
            for fc in range(2):
                mm(qp, hT_last[:, fc, :], cs["Wqe"][:, fc, :],
                   start=False, stop=(fc == 1))
            q = work.tile([P, DM], f16, name="qe")
            V.tensor_copy(out=q, in_=qp)
            sc = work.tile([P, H, TCACHE], f32, name="sc")
            tmp = work.tile([P, TCACHE, DH], f16, name="sctmp", tag="etmp")
            for h in range(H):
                V.tensor_tensor(out=tmp[:, 0:t, :],
                                in0=Kc[:, 0:t, h * DH:(h + 1) * DH],
                                in1=bc(q[:, h * DH:(h + 1) * DH], t),
                                op=OP.mult)
                V.tensor_reduce(out=sc[:, h, 0:t], in_=tmp[:, 0:t, :], axis=AX.X,
                                op=OP.add)
            S.activation(out=sc[:, :, 0:t], in_=sc[:, :, 0:t], func=AF.Exp)
            z = work.tile([P, H], f32, name="ze")
            V.tensor_reduce(out=z, in_=sc[:, :, 0:t], axis=AX.X, op=OP.add)
            V.reciprocal(out=z, in_=z)
            o = work.tile([P, DM], f16, name="oe")
            tmp2 = work.tile([P, DH, TCACHE], f16, name="otmp", tag="etmp")
            orow = work.tile([P, DH], f32, name="orow")
            for h in range(H):
                V.tensor_tensor(out=tmp2[:, :, 0:t], in0=Vc2[:, h, :, 0:t],
                                in1=bc(sc[:, h, 0:t], DH), op=OP.mult)
                V.tensor_reduce(out=orow, in_=tmp2[:, :, 0:t], axis=AX.X,
                                op=OP.add)
                V.tensor_scalar_mul(out=o[:, h * DH:(h + 1) * DH], in0=orow,
                                    scalar1=z[:, h:h + 1])
            oT = transpose128(o, 2, "oT")
            aop = ps_m([P, DM], "aoP")
            mm(aop, ones1, cs["bo"], start=True, stop=False)
            for fc in range(2):
                mm(aop, oT[:, fc, :], cs["Wo"][:, fc, :], start=False,
                   stop=(fc == 1))
            h1 = layer_norm(aop, h_nm_last, cs["g1"], cs["be1"], "1")
            h1T = transpose128(h1, 2, "h1T")
            zT = work.tile([P, 16, P], f16, name="zT", bufs=1)
            for half in range(2):
                zp = ps_big("SP")
                for s8 in range(8):
                    ffc = half * 8 + s8
                    mm(zp[:, s8, :], cs["b1f"][0:1, ffc * P:(ffc + 1) * P], ones1,
                       start=True, stop=False)
                    for fc in range(2):
                        mm(zp[:, s8, :], cs["W1"][:, fc, ffc * P:(ffc + 1) * P],
                           h1T[:, fc, :], start=False, stop=(fc == 1))
                S.activation(out=zT[:, half * 8:(half + 1) * 8, :], in_=zp,
                             func=AF.Relu)
            y2p = ps_m([P, DM], "y2P")
            mm(y2p, ones1, cs["b2f"], start=True, stop=False)
            for ffc in range(16):
                mm(y2p, zT[:, ffc, :], cs["W2"][:, ffc, :],
                   start=False, stop=(ffc == 15))
            ctx_ = layer_norm(y2p, h1, cs["g2"], cs["be2"], "2")
            ctxT = transpose128(ctx_, 2, "ctxT")
            return ctx_, ctxT

        # ------------------------------------------------------------------
        # Phase K: known steps with ONE batched AllGather
        # ------------------------------------------------------------------
        def tconv1_stage(midT, idx):
            kT, vloc = tconv1_proj(midT)
            binkv = dram.tile([2 * P, H * D], f16, name="binkv")
            boutkv = dram.tile([NC_ * 2 * P, H * D], f16, name="boutkv",
                               addr_space="Shared")
            nc.sync.dma_start(
                out=binkv[0:P, :].rearrange("p (c j) -> p c j", j=P), in_=kT)
            nc.sync.dma_start(out=binkv[P:2 * P, :], in_=vloc)
            nc.gpsimd.collective_compute(
                "AllGather", OP.bypass, replica_groups=RG,
                ins=[binkv[:]], outs=[boutkv[:]])
            gkv = boutkv.rearrange("(r t p) hd -> r t p hd", r=NC_, t=2)
            kst = big.tile([P, 8, 8, P], f16, name="kst", tag="kst", bufs=1)
            vall = big.tile([P, 8, H, D + 4], f16, name="vall", tag="vall", bufs=1)
            for r in range(NC_):
                ke = G if r < 4 else nc.scalar
                ve = nc.scalar if r < 4 else G
                ke.dma_start(out=kst[:, r, :, :],
                             in_=gkv[r, 0].rearrange("p (c j) -> p c j", j=P))
                ve.dma_start(out=vall[:, r, :, 0:D],
                             in_=gkv[r, 1].rearrange("p (h e) -> p h e", h=H))
            V.memset(vall[:, :, :, D:D + 1], 1.0)
            brpe = work.tile([1, DM], f16, name="brpe")
            nc.sync.dma_start(out=brpe, in_=d["brpe1"][idx])
            return (midT, kst, vall, brpe, idx)

        def tconv1_finish(midT, kst, vall, brpe, idx):
            return tconv1_attn(midT, kst, vall, idx, brpe)

        def tconv1_gather_attn(midT, idx):
            return tconv1_finish(*tconv1_stage(midT, idx))

        hT_last = None
        h_nm_last = None
        pending = None
        for i in range(N_KNOWN):
            kxTa = work.tile([AUG, N], f16, name="kxTa", bufs=3)
            nc.sync.dma_start(out=kxTa, in_=d["kxTaug_all"][i])
            kxa = work.tile([P, 8, AUG], f16, name="kxa", bufs=3)
            nc.sync.dma_start(out=kxa, in_=d["kxaug_all"][i])
            kxTl = work.tile([AUG, P], f16, name="kxTl", bufs=3)
            nc.sync.dma_start(out=kxTl, in_=d["kxTaug_loc"][i])
            midT = tconv0(lambda cc, _t=kxTa: _t[:, cc * P:(cc + 1) * P],
                          lambda cc, _t=kxa: _t[:, cc, :], kxTl)
            staged = tconv1_stage(midT, i)
            if pending is not None:
                h_nm_last, hT_last = tconv1_finish(*pending)
                cache_update(pending[-1], hT_last)
            pending = staged
        h_nm_last, hT_last = tconv1_finish(*pending)
        cache_update(pending[-1], hT_last)

        # ------------------------------------------------------------------
        # Phase G: autoregressive generation
        # ------------------------------------------------------------------
        for t in range(K, K + N_GEN):
            ctx_, ctxT = enc(t, hT_last, h_nm_last)
            wdp = ps_m([P, F - 2], "wdP")
            mm(wdp, ones1, cs["bd"], start=True, stop=False)
            for fc in range(2):
                mm(wdp, ctxT[:, fc, :], cs["Wd"][:, fc, :],
                   start=False, stop=(fc == 1))
            xn32 = work.tile([P, F - 2], f32, name="xn32")
            V.tensor_copy(out=xn32, in_=wdp)
            nc.sync.dma_start(out=out_d[t - K, :, 0:2], in_=d["latlon32"])
            nc.sync.dma_start(out=out_d[t - K, :, 2:F], in_=xn32)
            if t == K + N_GEN - 1 or t == NG - 1:
                break
            V.tensor_copy(out=xaug[:, 0:F - 2], in_=wdp)
            tp = ps_m([F - 2, P], "ptp", dtype=f16)
            mm(tp, xaug[:, 0:F - 2], idn, start=True, stop=True, is_transpose=True)
            V.tensor_copy(out=xTaug[0:F - 2, :], in_=tp)
            gin = dram.tile([1, 2 * AUG * P], f16, name="g_in")
            gout = dram.tile([NC_, 2 * AUG * P], f16, name="g_out",
                             addr_space="Shared")
            G.dma_start(
                out=gin[0, 0:AUG * P].rearrange("(p j) -> p j", p=AUG), in_=xTaug)
            G.dma_start(
                out=gin[0, AUG * P:2 * AUG * P].rearrange("(p j) -> p j", p=P),
                in_=xaug)
            nc.gpsimd.collective_compute(
                "AllGather", OP.bypass, replica_groups=RG,
                ins=[gin[:]], outs=[gout[:]])
            xTa_all = work.tile([AUG, 8, P], f16, name="xTa_all")
            G.dma_start(
                out=xTa_all,
                in_=gout[:, 0:AUG * P].rearrange("r (p j) -> p r j", p=AUG))
            xa_all = work.tile([P, 8, AUG], f16, name="xa_all")
            G.dma_start(
                out=xa_all,
                in_=gout[:, AUG * P:2 * AUG * P].rearrange("r (p j) -> p r j", p=P))
            midT = tconv0(lambda cc, _t=xTa_all: _t[:, cc, :],
                          lambda cc, _t=xa_all: _t[:, cc, :], xTaug)
            h_nm_last, hT_last = tconv1_gather_attn(midT, t)
            cache_update(t, hT_last)

    nc.finalize()
    return nc


# ----------------------------------------------------------------------------
# Host-side preprocessing
# ----------------------------------------------------------------------------
def prep_in_maps(inputs):
    f32 = np.float32
    f16 = np.float16
    g = {k: np.asarray(v) for k, v in inputs.items()}
    kx = g["known_x"].astype(f32)                       # [10, 1024, 10]
    ei = g["edge_index"].astype(np.int64)

    Cnt = np.zeros((N, N), f32)
    np.add.at(Cnt, (ei[0], ei[1]), 1.0)
    LT = np.where(Cnt > 0, np.log(np.maximum(Cnt, 1.0)), _NEG).astype(f32)

    isd = f32(1.0 / np.sqrt(D))
    PERM = [2, 3, 4, 5, 6, 7, 8, 9, 0, 1, 10]
    Wq0a = (np.vstack([g["Wq0"], g["bq0"][None]]).astype(f32) * isd)[PERM]
    Wk0a = np.vstack([g["Wk0"], g["bk0"][None]]).astype(f32)[PERM]
    A = np.stack([(Wk0a[:, h * D:(h + 1) * D] @ Wq0a[:, h * D:(h + 1) * D].T)
                  for h in range(H)])                                # [4, 11, 11]
    AT = A.transpose(0, 2, 1).transpose(1, 0, 2).copy()              # [11, 4, 11]

    kxaug = np.concatenate([kx, np.ones((K, N, 1), f32)], axis=2)[:, :, PERM]
    kxTaug = kxaug.transpose(0, 2, 1).copy()                         # [10, 11, 1024]

    ide = f32(1.0 / np.sqrt(DH))
    Wqkv, bqkv = g["Wqkv"].astype(f32), g["bqkv"].astype(f32)

    def w2t(w, nch):
        m = w.shape[1]
        return np.ascontiguousarray(
            np.asarray(w, f32).reshape(nch, P, m).transpose(1, 0, 2))

    Wv0a = np.vstack([g["Wv0"], g["bv0"][None]]).astype(f32)[PERM]   # [11, 1024]
    Wv0az = np.zeros((AUG, H, D + 1), f32)
    for h in range(H):
        Wv0az[:, h, 0:D] = Wv0a[:, h * D:(h + 1) * D]
    Wv0az[10, :, D] = 1.0      # e10 column (ones-row index under PERM) -> z

    common = {
        "AT": AT.astype(f16),
        "Wv0az": Wv0az.astype(f16),
        "Wr0a": np.vstack([g["Wr0"], g["br0"][None]])[PERM].astype(f16),
        "Wq1s": w2t(np.asarray(g["Wq1"], f32) * isd, 2).astype(f16),
        "Wk1": w2t(g["Wk1"], 2).astype(f16),
        "Wv1": w2t(g["Wv1"], 2).astype(f16),
        "bq1b": np.ascontiguousarray(np.broadcast_to(
            (np.asarray(g["bq1"], f32) * isd).reshape(8, P).T[:, :, None],
            (P, 8, P))).astype(f32),
        "bk1b": np.ascontiguousarray(np.broadcast_to(
            np.asarray(g["bk1"], f32).reshape(8, P).T[:, :, None],
            (P, 8, P))).astype(f32),
        "bv1": np.asarray(g["bv1"], f16)[None],
        "Wr1": w2t(g["Wr1"], 2).astype(f16),
        "brpe1": (np.asarray(g["br1"], f32)[None]
                  + np.asarray(g["pe"], f32))[:, None, :].astype(f16),
        "Wqe": w2t(Wqkv[:, 0:DM] * ide, 2).astype(f16),
        "Wke": w2t(Wqkv[:, DM:2 * DM], 2).astype(f16),
        "Wve": w2t(Wqkv[:, 2 * DM:], 2).astype(f16),
        "bqe": (bqkv[0:DM] * ide)[None].astype(f16),
        "bke": bqkv[DM:2 * DM][None].astype(f16),
        "bve": bqkv[2 * DM:][None].astype(f16),
        "Wo": w2t(g["Wo"], 2).astype(f16),
        "bo": np.asarray(g["bo"], f16)[None],
        "W1": w2t(g["W1"], 2).astype(f16),
        "b1f": np.asarray(g["b1f"], f16)[None],
        "W2": w2t(g["W2"], 16).astype(f16),
        "b2f": np.asarray(g["b2f"], f16)[None],
        "Wd": w2t(g["Wd"], 2).astype(f16),
        "bd": np.asarray(g["bd"], f16)[None],
        "g1": np.asarray(g["g1"], f32)[None],
        "be1": np.asarray(g["be1"], f32)[None],
        "g2": np.asarray(g["g2"], f32)[None],
        "be2": np.asarray(g["be2"], f32)[None],
        "idn": np.eye(P, dtype=f16),
        "ones1": np.ones((1, P), f16),
        "kxTaug_all": kxTaug.astype(f16),
        "kxaug_all": np.ascontiguousarray(
            kxaug.reshape(K, 8, P, AUG).transpose(0, 2, 1, 3)).astype(f16),
    }
    in_maps = []
    for c in range(NC_):
        sl = slice(P * c, P * (c + 1))
        m = dict(common)
        m["lt"] = np.ascontiguousarray(
            LT[:, sl].reshape(8, P, P).transpose(1, 0, 2)).astype(f16)
        m["latlon32"] = np.ascontiguousarray(kx[K - 1, sl, 0:2]).astype(f32)
        ll = kx[K - 1, sl, 0:2].astype(f32)
        xti = np.zeros((AUG, P), f32); xti[8:10] = ll.T; xti[10] = 1.0
        m["xTaug_init"] = xti.astype(f16)
        xai = np.zeros((P, AUG), f32); xai[:, 8:10] = ll; xai[:, 10] = 1.0
        m["xaug_init"] = xai.astype(f16)
        m["kxTaug_loc"] = np.ascontiguousarray(kxTaug[:, :, sl]).astype(f16)
        in_maps.append(m)
    return in_maps


_CACHED = {}


def run(inputs, trace=False, trace_kwargs=None):
    from concourse import bass_utils
    if "nc" not in _CACHED:
        _CACHED["nc"] = build_bass()
    in_maps = prep_in_maps(inputs)
    res = bass_utils.run_bass_kernel_spmd(
        _CACHED["nc"], in_maps, core_ids=list(range(NC_)), trace=trace,
        **(trace_kwargs or {}))
    out = np.concatenate([res.results[c]["out"] for c in range(NC_)], axis=1)
    return out.astype(np.float32), res


def kernel(**inputs):
    out, _ = run(inputs, trace=False)
    return out


# revision 22
# speedup vs baseline: 1.1823x; 1.1646x over previous
"""Trainium2 Bass kernel for nn_AdaptiveConditionedGraphTransformer.

Strategy (8 NeuronCores, data-parallel over nodes, 128 nodes/core):
- Graph edge-attention (PyG TransformerConv) computed DENSELY per core as
  [src=1024, dst=128] score matrices on TensorE; duplicate edges + masking
  handled exactly via a host-precomputed ln(count) additive mask injected
  into PSUM with an identity matmul inside the QK^T accumulation group.
- Layer-0 tconv (fin=10) uses a rank-11 factorization: S = x_aug A x_aug^T
  with A = Wk_aug Wq_aug^T/16 precomputed on host -> no K/V materialization
  and only a 5.5KB/rank AllGather of x_aug per autoregressive step.
- Layer-1 tconv computes K^T/V locally; the 10 independent known steps share
  ONE batched 5MB AllGather; autoregressive steps use split K/V AllGathers
  so score matmuls overlap the V gather.
- Softmax denominators ride along in the aggregation matmuls (ones column
  appended to V / e10 column appended to Wv0aug) -- no separate z matmuls.
- Temporal transformer layer: exact KV-cache, last-query only, on VectorE.
- LayerNorm rsqrt via exp(-0.5*ln(var+eps)) so ScalarE stays on the single
  natural_log_exp table set the whole kernel.
- All matmul inputs fp16 (fp32 PSUM accumulate); softmax/LN math fp32.

kernel(**inputs) takes FULL inputs, shards internally, returns FULL output.
"""
import os
import sys

import numpy as np

sys.path.insert(0, "/opt/trn_rl_repo")

N, E, F = 1024, 16384, 10
DM, H = 256, 4
D = 256
NG, K = 20, 10
FF = 2048
DH = DM // H
NC_ = 8
P = N // NC_          # 128 nodes per core
AUG = F + 1           # 11
TCACHE = NG - 1       # 19 cache slots

N_KNOWN = int(os.environ.get("GT_KNOWN", "10"))
N_GEN = int(os.environ.get("GT_GEN", "10"))

_NEG = -30000.0


# ----------------------------------------------------------------------------
# Device program
# ----------------------------------------------------------------------------
def build_bass():
    import concourse.bass as bass
    import concourse.tile as tile
    from concourse import bacc, mybir

    f16 = mybir.dt.float16
    f32 = mybir.dt.float32
    AF = mybir.ActivationFunctionType
    OP = mybir.AluOpType
    AX = mybir.AxisListType

    nc = bacc.Bacc("TRN2", target_bir_lowering=False, debug=False, num_devices=NC_)
    RG = [list(range(NC_))]

    def din(name, shape, dtype):
        return nc.dram_tensor(name, list(shape), dtype, kind="ExternalInput").ap()

    d = {}
    d["lt"] = din("lt", [P, 8, P], f16)
    d["latlon32"] = din("latlon32", [P, 2], f32)
    d["AT"] = din("AT", [AUG, H, AUG], f16)
    d["Wv0az"] = din("Wv0az", [AUG, H, D + 1], f16)
    d["Wr0a"] = din("Wr0a", [AUG, DM], f16)
    d["kxTaug_all"] = din("kxTaug_all", [K, AUG, N], f16)
    d["kxaug_all"] = din("kxaug_all", [K, P, 8, AUG], f16)
    d["kxTaug_loc"] = din("kxTaug_loc", [K, AUG, P], f16)
    d["Wq1s"] = din("Wq1s", [P, 2, H * D], f16)
    d["Wk1"] = din("Wk1", [P, 2, H * D], f16)
    d["Wv1"] = din("Wv1", [P, 2, H * D], f16)
    d["bq1b"] = din("bq1b", [P, 8, P], f32)
    d["bk1b"] = din("bk1b", [P, 8, P], f32)
    d["bv1"] = din("bv1", [1, H * D], f16)
    d["Wr1"] = din("Wr1", [P, 2, DM], f16)
    d["brpe1"] = din("brpe1", [NG, 1, DM], f16)
    d["Wqe"] = din("Wqe", [P, 2, DM], f16)
    d["Wke"] = din("Wke", [P, 2, DM], f16)
    d["Wve"] = din("Wve", [P, 2, DM], f16)
    d["Wo"] = din("Wo", [P, 2, DM], f16)
    d["bqe"] = din("bqe", [1, DM], f16)
    d["bke"] = din("bke", [1, DM], f16)
    d["bve"] = din("bve", [1, DM], f16)
    d["bo"] = din("bo", [1, DM], f16)
    d["W1"] = din("W1", [P, 2, FF], f16)
    d["b1f"] = din("b1f", [1, FF], f16)
    d["W2"] = din("W2", [P, 16, DM], f16)
    d["b2f"] = din("b2f", [1, DM], f16)
    d["Wd"] = din("Wd", [P, 2, F - 2], f16)
    d["bd"] = din("bd", [1, F - 2], f16)
    d["g1"] = din("g1", [1, DM], f32)
    d["be1"] = din("be1", [1, DM], f32)
    d["g2"] = din("g2", [1, DM], f32)
    d["be2"] = din("be2", [1, DM], f32)
    d["xTaug_init"] = din("xTaug_init", [AUG, P], f16)
    d["xaug_init"] = din("xaug_init", [P, AUG], f16)
    d["idn"] = din("idn", [P, P], f16)
    d["ones1"] = din("ones1", [1, P], f16)

    out_d = nc.dram_tensor("out", [N_GEN, P, F], f32, kind="ExternalOutput").ap()

    mm = nc.tensor.matmul
    V = nc.vector
    S = nc.scalar
    G = nc.gpsimd

    def bc(ap, ins_size):
        """Insert a stride-0 dim before the last free dim: [p, n] -> [p, m, n]."""
        return bass.AP(tensor=ap.tensor, offset=ap.offset,
                       ap=[list(ap.ap[0]), [0, ins_size], list(ap.ap[1])])

    from contextlib import ExitStack
    with tile.TileContext(nc) as tc, ExitStack() as ctx:
        consts = ctx.enter_context(tc.tile_pool(name="consts", bufs=1))
        state = ctx.enter_context(tc.tile_pool(name="state", bufs=1))
        work = ctx.enter_context(tc.tile_pool(name="work", bufs=2))
        big = ctx.enter_context(tc.tile_pool(name="big", bufs=1))
        psA = ctx.enter_context(tc.tile_pool(name="psA", bufs=2, space="PSUM"))
        psS = ctx.enter_context(tc.tile_pool(name="psS", bufs=2, space="PSUM"))
        psM = ctx.enter_context(tc.tile_pool(name="psM", bufs=2, space="PSUM"))
        dram = ctx.enter_context(tc.tile_pool(name="dram", bufs=2, space="DRAM"))

        def ps_big(name, shape=(P, 8, P)):
            return psA.tile(list(shape), f32, name=name, tag="SP")

        def ps_m(shape, name, dtype=None):
            return psM.tile(list(shape), dtype or f32, name=name, tag="mP")

        cs = {}
        for name, ap in d.items():
            if name in ("g1", "be1", "g2", "be2", "kxTaug_all", "kxaug_all",
                        "kxTaug_loc", "xTaug_init", "xaug_init", "latlon32"):
                continue
            if name == "brpe1":
                continue
            t = consts.tile(list(ap.shape), ap.dtype, name=f"c_{name}")
            nc.sync.dma_start(out=t, in_=ap)
            cs[name] = t
        for name in ("g1", "be1", "g2", "be2"):
            t = consts.tile([P, DM], f32, name=f"c_{name}")
            nc.sync.dma_start(out=t, in_=d[name].to_broadcast((P, DM)))
            cs[name] = t
        latlon32 = consts.tile([P, 2], f32, name="latlon32")
        nc.sync.dma_start(out=latlon32, in_=d["latlon32"])

        idn = cs["idn"]; ones1 = cs["ones1"]
        lt = cs["lt"]
        eps1 = consts.tile([P, 1], f32, name="eps1")
        V.memset(eps1, 1e-5)

        Kc = state.tile([P, TCACHE, DM], f16, name="Kc")
        Vc2 = state.tile([P, H, DH, TCACHE], f16, name="Vc2")
        xTaug = state.tile([AUG, P], f16, name="xTaug")
        xaug = state.tile([P, AUG], f16, name="xaug")
        nc.sync.dma_start(out=xTaug, in_=d["xTaug_init"])
        nc.sync.dma_start(out=xaug, in_=d["xaug_init"])

        # ------------------------------------------------------------------
        def transpose128(src_ap, n_chunks, name):
            dst = work.tile([P, n_chunks, P], f16, name=name,
                            bufs=(4 if name == "midT" else None))
            for fc in range(n_chunks):
                pt = ps_m([P, P], "ptp", dtype=f16)
                mm(pt, src_ap[:, fc * P:(fc + 1) * P], idn, start=True, stop=True,
                   is_transpose=True)
                V.tensor_copy(out=dst[:, fc, :], in_=pt)
            return dst

        def combine(agg01, agg23, rootP, name):
            """sum_h agg_h/(4 z_h) + rootP; z_h rides in agg col 256."""
            zi = work.tile([P, H], f32, name=f"zi_{name}")
            V.tensor_scalar(out=zi[:, 0:2],
                            in0=agg01[:, :, 256:257].rearrange("p h x -> p (h x)"),
                            scalar1=4.0, scalar2=4e-16, op0=OP.mult, op1=OP.add)
            V.tensor_scalar(out=zi[:, 2:4],
                            in0=agg23[:, :, 256:257].rearrange("p h x -> p (h x)"),
                            scalar1=4.0, scalar2=4e-16, op0=OP.mult, op1=OP.add)
            V.reciprocal(out=zi, in_=zi)
            t0 = work.tile([P, DM], f32, name=f"cmb_{name}")
            V.tensor_scalar_mul(out=t0, in0=agg01[:, 0, 0:256], scalar1=zi[:, 0:1])
            V.scalar_tensor_tensor(out=t0, in0=agg01[:, 1, 0:256],
                                   scalar=zi[:, 1:2], in1=t0,
                                   op0=OP.mult, op1=OP.add)
            V.scalar_tensor_tensor(out=t0, in0=agg23[:, 0, 0:256],
                                   scalar=zi[:, 2:3], in1=t0,
                                   op0=OP.mult, op1=OP.add)
            V.scalar_tensor_tensor(out=t0, in0=agg23[:, 1, 0:256],
                                   scalar=zi[:, 3:4], in1=t0,
                                   op0=OP.mult, op1=OP.add)
            out_nm = work.tile([P, DM], f16, name=f"nm_{name}")
            V.tensor_tensor(out=out_nm, in0=rootP, in1=t0, op=OP.add)
            return out_nm

        def tconv0(xTa_all_fn, xa_all_fn, xTa_loc):
            """Rank-11 layer-0 tconv."""
            MpP = ps_m([AUG, H, P], "MpP")
            for h in range(H):
                mm(MpP[:, h, :], cs["AT"][:, h, :], xTa_loc, start=True, stop=True)
            Mp = work.tile([AUG, H, P], f16, name="Mp")
            V.tensor_copy(out=Mp, in_=MpP)
            PT = big.tile([P, H, 8, P], f16, name="PT", tag="PT", bufs=2)
            GTP = ps_m([AUG, H, P], "GTP")
            for cc in range(8):
                sp = psS.tile([P, H, P], f32, name="Scc", tag="Scc")
                mm(sp, idn, bc(lt[:, cc, :], H), start=True, stop=False)
                for h in range(H):
                    mm(sp[:, h, :], xTa_all_fn(cc), Mp[:, h, :],
                       start=False, stop=True)
                S.activation(out=PT[:, :, cc, :], in_=sp, func=AF.Exp)
            for cc in range(8):
                mm(GTP, xa_all_fn(cc), PT[:, :, cc, :],
                   start=(cc == 0), stop=(cc == 7))
            GT = work.tile([AUG, H, P], f16, name="GT")
            V.tensor_copy(out=GT, in_=GTP)
            agg01 = ps_big("agg01", (P, 2, 512))
            agg23 = ps_big("agg23", (P, 2, 512))
            for h in range(H):
                dst = (agg01, agg23)[h // 2][:, h % 2, 0:D + 1]
                mm(dst, GT[:, h, :], cs["Wv0az"][:, h, :], start=True, stop=True)
            rootP = ps_m([P, DM], "rootP")
            mm(rootP, xTa_loc, cs["Wr0a"], start=True, stop=True)
            mid = combine(agg01, agg23, rootP, "t0")
            return transpose128(mid, 2, "midT")

        def proj_T(midT, W, b, name):
            dst_sb = work.tile([P, 8, P], f16, name=name)
            pp = ps_big("SP")
            for cc in range(8):
                for fc in range(2):
                    mm(pp[:, cc, :], W[:, fc, cc * P:(cc + 1) * P],
                       midT[:, fc, :], start=(fc == 0), stop=(fc == 1))
            V.tensor_tensor(out=dst_sb, in0=pp, in1=b, op=OP.add)
            return dst_sb

        def tconv1_proj(midT):
            """Local K/V projections: k1T [128,8,128] f16, v [128,1024] f16."""
            kT = proj_T(midT, cs["Wk1"], cs["bk1b"], "k1T")
            vloc = work.tile([P, H * D], f16, name="v1loc")
            for hf in range(2):
                vp = ps_m([P, 512], "vP")
                mm(vp, ones1, cs["bv1"][0:1, hf * 512:(hf + 1) * 512],
                   start=True, stop=False)
                for fc in range(2):
                    mm(vp, midT[:, fc, :], cs["Wv1"][:, fc, hf * 512:(hf + 1) * 512],
                       start=False, stop=(fc == 1))
                V.tensor_copy(out=vloc[:, hf * 512:(hf + 1) * 512], in_=vp)
            return kT, vloc

        def tconv1_attn(midT, kst, vall, idx, brpe):
            """Dense edge attention from gathered K/V. Returns (h_nm, hT)."""
            qT = proj_T(midT, cs["Wq1s"], cs["bq1b"], "q1T")
            PT = big.tile([P, H, 8, P], f16, name="PT", tag="PT", bufs=2)
            for cc in range(8):
                sp = psS.tile([P, H, P], f32, name="Scc", tag="Scc")
                mm(sp, idn, bc(lt[:, cc, :], H), start=True, stop=False)
                for h in range(H):
                    for dc in range(2):
                        mm(sp[:, h, :], kst[:, cc, 2 * h + dc, :],
                           qT[:, 2 * h + dc, :], start=False, stop=(dc == 1))
                S.activation(out=PT[:, :, cc, :], in_=sp, func=AF.Exp)
            agg01 = ps_big("agg01", (P, 2, 512))
            agg23 = ps_big("agg23", (P, 2, 512))
            for h in range(H):
                dst = (agg01, agg23)[h // 2][:, h % 2, 0:D + 1]
                for cc in range(8):
                    mm(dst, PT[:, h, cc, :], vall[:, cc, h, 0:D + 1],
                       start=(cc == 0), stop=(cc == 7))
            rootP = ps_m([P, DM], "rootP")
            mm(rootP, ones1, brpe, start=True, stop=False)
            for fc in range(2):
                mm(rootP, midT[:, fc, :], cs["Wr1"][:, fc, :],
                   start=False, stop=(fc == 1))
            h_nm = combine(agg01, agg23, rootP, "t1")
            hT = transpose128(h_nm, 2, "hT")
            return h_nm, hT

        def cache_update(slot, hT):
            for W, b, which in ((cs["Wke"], cs["bke"], "k"),
                                (cs["Wve"], cs["bve"], "v")):
                pp = ps_m([P, DM], "cuP")
                mm(pp, ones1, b, start=True, stop=False)
                for fc in range(2):
                    mm(pp, hT[:, fc, :], W[:, fc, :], start=False, stop=(fc == 1))
                if which == "k":
                    V.tensor_copy(out=Kc[:, slot, :], in_=pp)
                else:
                    V.tensor_copy(out=Vc2[:, :, :, slot],
                                  in_=pp.rearrange("p (h e) -> p h e", h=H))

        def layer_norm(x_f32_psum, resid_f16, g, be, name):
            t1 = work.tile([P, DM], f32, name=f"ln_t1_{name}")
            V.tensor_tensor(out=t1, in0=x_f32_psum, in1=resid_f16, op=OP.add)
            st = work.tile([P, 6], f32, name=f"ln_st_{name}")
            V.bn_stats(out=st, in_=t1)
            mv = work.tile([P, 2], f32, name=f"ln_mv_{name}")
            V.bn_aggr(out=mv, in_=st)
            vv = work.tile([P, 1], f32, name=f"ln_vv_{name}")
            V.tensor_scalar_add(out=vv, in0=mv[:, 1:2], scalar1=1e-5)
            rs = work.tile([P, 1], f32, name=f"ln_rs_{name}")
            rsi = rs.bitcast(mybir.dt.int32)
            V.tensor_scalar(out=rsi, in0=vv.bitcast(mybir.dt.int32),
                            scalar1=1, scalar2=None, op0=OP.arith_shift_right)
            V.tensor_scalar(out=rsi, in0=rsi, scalar1=-1, scalar2=0x5F3759DF,
                            op0=OP.mult, op1=OP.add)
            t_n = work.tile([P, 1], f32, name=f"ln_nt_{name}")
            for _ in range(2):
                V.tensor_tensor(out=t_n, in0=rs, in1=rs, op=OP.mult)
                V.tensor_tensor(out=t_n, in0=t_n, in1=vv, op=OP.mult)
                V.tensor_scalar(out=t_n, in0=t_n, scalar1=-0.5, scalar2=1.5,
                                op0=OP.mult, op1=OP.add)
                V.tensor_tensor(out=rs, in0=rs, in1=t_n, op=OP.mult)
            V.tensor_scalar(out=t1, in0=t1, scalar1=mv[:, 0:1], scalar2=rs,
                            op0=OP.subtract, op1=OP.mult)
            V.tensor_tensor(out=t1, in0=t1, in1=g, op=OP.mult)
            o = work.tile([P, DM], f16, name=f"ln_o_{name}")
            V.tensor_tensor(out=o, in0=t1, in1=be, op=OP.add)
            return o

        def enc(t, hT_last, h_nm_last):
            qp = ps_m([P, DM], "qeP")
            mm(qp, ones1, cs["bqe"], start=True, stop=False)
            for fc in range(2):
                mm(qp, hT_last[:, fc, :], cs["Wqe"][:, fc, :],
                   start=False, stop=(fc == 1))
            q = work.tile([P, DM], f16, name="qe")
            V.tensor_copy(out=q, in_=qp)
            sc = work.tile([P, H, TCACHE], f16, name="sc")
            tmp = work.tile([P, TCACHE, DH], f16, name="sctmp", tag="etmp")
            for h in range(H):
                V.tensor_tensor(out=tmp[:, 0:t, :],
                                in0=Kc[:, 0:t, h * DH:(h + 1) * DH],
                                in1=bc(q[:, h * DH:(h + 1) * DH], t),
                                op=OP.mult)
                with nc.allow_low_precision("f16 attn scores, |s|<~4"):
                    V.tensor_reduce(out=sc[:, h, 0:t], in_=tmp[:, 0:t, :],
                                    axis=AX.X, op=OP.add)
            S.activation(out=sc[:, :, 0:t], in_=sc[:, :, 0:t], func=AF.Exp)
            z = work.tile([P, H], f32, name="ze")
            V.tensor_reduce(out=z, in_=sc[:, :, 0:t], axis=AX.X, op=OP.add)
            V.reciprocal(out=z, in_=z)
            o = work.tile([P, DM], f16, name="oe")
            tmp2 = work.tile([P, DH, TCACHE], f16, name="otmp", tag="etmp")
            orow = work.tile([P, DH], f32, name="orow")
            for h in range(H):
                V.tensor_tensor(out=tmp2[:, :, 0:t], in0=Vc2[:, h, :, 0:t],
                                in1=bc(sc[:, h, 0:t], DH), op=OP.mult)
                V.tensor_reduce(out=orow, in_=tmp2[:, :, 0:t], axis=AX.X,
                                op=OP.add)
                V.tensor_scalar_mul(out=o[:, h * DH:(h + 1) * DH], in0=orow,
                                    scalar1=z[:, h:h + 1])
            oT = transpose128(o, 2, "oT")
            aop = ps_m([P, DM], "aoP")
            mm(aop, ones1, cs["bo"], start=True, stop=False)
            for fc in range(2):
                mm(aop, oT[:, fc, :], cs["Wo"][:, fc, :], start=False,
                   stop=(fc == 1))
            h1 = layer_norm(aop, h_nm_last, cs["g1"], cs["be1"], "1")
            h1T = transpose128(h1, 2, "h1T")
            zT = work.tile([P, 16, P], f16, name="zT", bufs=1)
            for half in range(2):
                zp = ps_big("SP")
                for s8 in range(8):
                    ffc = half * 8 + s8
                    mm(zp[:, s8, :], cs["b1f"][0:1, ffc * P:(ffc + 1) * P], ones1,
                       start=True, stop=False)
                    for fc in range(2):
                        mm(zp[:, s8, :], cs["W1"][:, fc, ffc * P:(ffc + 1) * P],
                           h1T[:, fc, :], start=False, stop=(fc == 1))
                V.tensor_scalar_max(out=zT[:, half * 8:(half + 1) * 8, :],
                                    in0=zp, scalar1=0.0)
            y2p = ps_m([P, DM], "y2P")
            mm(y2p, ones1, cs["b2f"], start=True, stop=False)
            for ffc in range(16):
                mm(y2p, zT[:, ffc, :], cs["W2"][:, ffc, :],
                   start=False, stop=(ffc == 15))
            ctx_ = layer_norm(y2p, h1, cs["g2"], cs["be2"], "2")
            ctxT = transpose128(ctx_, 2, "ctxT")
            return ctx_, ctxT

        # ------------------------------------------------------------------
        # Phase K: known steps with ONE batched AllGather
        # ------------------------------------------------------------------
        def tconv1_stage(midT, idx):
            kT, vloc = tconv1_proj(midT)
            binkv = dram.tile([2 * P, H * D], f16, name="binkv")
            boutkv = dram.tile([NC_ * 2 * P, H * D], f16, name="boutkv",
                               addr_space="Shared")
            bk2 = binkv[0:P, :].rearrange("p (c j) -> p c j", j=P)
            for qd in range(4):
                nc.sync.dma_start(out=bk2[:, 2 * qd:2 * qd + 2, :],
                                  in_=kT[:, 2 * qd:2 * qd + 2, :])
                G.dma_start(out=binkv[P:2 * P, 256 * qd:256 * (qd + 1)],
                            in_=vloc[:, 256 * qd:256 * (qd + 1)])
            nc.gpsimd.collective_compute(
                "AllGather", OP.bypass, replica_groups=RG,
                ins=[binkv[:]], outs=[boutkv[:]])
            gkv = boutkv.rearrange("(r t p) hd -> r t p hd", r=NC_, t=2)
            kst = big.tile([P, 8, 8, P], f16, name="kst", tag="kst", bufs=1)
            vall = big.tile([P, 8, H, D + 4], f16, name="vall", tag="vall", bufs=1)
            for r in range(NC_):
                ke = G if r < 4 else nc.sync
                ke.dma_start(out=kst[:, r, :, :],
                             in_=gkv[r, 0].rearrange("p (c j) -> p c j", j=P))
            for r in range(NC_):
                nc.sync.dma_start(out=vall[:, r, :, 0:D],
                                  in_=gkv[r, 1].rearrange("p (h e) -> p h e", h=H))
            V.memset(vall[:, :, :, D:D + 1], 1.0)
            brpe = work.tile([1, DM], f16, name="brpe")
            nc.sync.dma_start(out=brpe, in_=d["brpe1"][idx])
            return (midT, kst, vall, brpe, idx)

        def tconv1_finish(midT, kst, vall, brpe, idx):
            return tconv1_attn(midT, kst, vall, idx, brpe)

        def tconv1_gather_attn(midT, idx):
            return tconv1_finish(*tconv1_stage(midT, idx))

        hT_last = None
        h_nm_last = None
        pending = None
        for i in range(N_KNOWN):
            kxTa = work.tile([AUG, N], f16, name="kxTa", bufs=3)
            nc.sync.dma_start(out=kxTa, in_=d["kxTaug_all"][i])
            kxa = work.tile([P, 8, AUG], f16, name="kxa", bufs=3)
            nc.sync.dma_start(out=kxa, in_=d["kxaug_all"][i])
            kxTl = work.tile([AUG, P], f16, name="kxTl", bufs=3)
            nc.sync.dma_start(out=kxTl, in_=d["kxTaug_loc"][i])
            midT = tconv0(lambda cc, _t=kxTa: _t[:, cc * P:(cc + 1) * P],
                          lambda cc, _t=kxa: _t[:, cc, :], kxTl)
            staged = tconv1_stage(midT, i)
            if pending is not None:
                h_nm_last, hT_last = tconv1_finish(*pending)
                cache_update(pending[-1], hT_last)
            pending = staged
        h_nm_last, hT_last = tconv1_finish(*pending)
        cache_update(pending[-1], hT_last)

        # ------------------------------------------------------------------
        # Phase G: autoregressive generation
        # ------------------------------------------------------------------
        for t in range(K, K + N_GEN):
            ctx_, ctxT = enc(t, hT_last, h_nm_last)
            wdp = ps_m([P, F - 2], "wdP")
            mm(wdp, ones1, cs["bd"], start=True, stop=False)
            for fc in range(2):
                mm(wdp, ctxT[:, fc, :], cs["Wd"][:, fc, :],
                   start=False, stop=(fc == 1))
            xn32 = work.tile([P, F - 2], f32, name="xn32")
            V.tensor_copy(out=xn32, in_=wdp)
            nc.sync.dma_start(out=out_d[t - K, :, 0:2], in_=d["latlon32"])
            nc.sync.dma_start(out=out_d[t - K, :, 2:F], in_=xn32)
            if t == K + N_GEN - 1 or t == NG - 1:
                break
            V.tensor_copy(out=xaug[:, 0:F - 2], in_=wdp)
            tp = ps_m([F - 2, P], "ptp", dtype=f16)
            mm(tp, xaug[:, 0:F - 2], idn, start=True, stop=True, is_transpose=True)
            V.tensor_copy(out=xTaug[0:F - 2, :], in_=tp)
            gin = dram.tile([1, 2 * AUG * P], f16, name="g_in")
            gout = dram.tile([NC_, 2 * AUG * P], f16, name="g_out",
                             addr_space="Shared")
            G.dma_start(
                out=gin[0, 0:AUG * P].rearrange("(p j) -> p j", p=AUG), in_=xTaug)
            G.dma_start(
                out=gin[0, AUG * P:2 * AUG * P].rearrange("(p j) -> p j", p=P),
                in_=xaug)
            nc.gpsimd.collective_compute(
                "AllGather", OP.bypass, replica_groups=RG,
                ins=[gin[:]], outs=[gout[:]])
            xTa_all = work.tile([AUG, 8, P], f16, name="xTa_all")
            G.dma_start(
                out=xTa_all,
                in_=gout[:, 0:AUG * P].rearrange("r (p j) -> p r j", p=AUG))
            xa_all = work.tile([P, 8, AUG], f16, name="xa_all")
            G.dma_start(
                out=xa_all,
                in_=gout[:, AUG * P:2 * AUG * P].rearrange("r (p j) -> p r j", p=P))
            midT = tconv0(lambda cc, _t=xTa_all: _t[:, cc, :],
                          lambda cc, _t=xa_all: _t[:, cc, :], xTaug)
            h_nm_last, hT_last = tconv1_gather_attn(midT, t)
            cache_update(t, hT_last)

    nc.finalize()
    return nc


# ----------------------------------------------------------------------------
# Host-side preprocessing
# ----------------------------------------------------------------------------
def prep_in_maps(inputs):
    f32 = np.float32
    f16 = np.float16
    g = {k: np.asarray(v) for k, v in inputs.items()}
    kx = g["known_x"].astype(f32)                       # [10, 1024, 10]
    ei = g["edge_index"].astype(np.int64)

    Cnt = np.zeros((N, N), f32)
    np.add.at(Cnt, (ei[0], ei[1]), 1.0)
    LT = np.where(Cnt > 0, np.log(np.maximum(Cnt, 1.0)), _NEG).astype(f32)

    isd = f32(1.0 / np.sqrt(D))
    PERM = [2, 3, 4, 5, 6, 7, 8, 9, 0, 1, 10]
    Wq0a = (np.vstack([g["Wq0"], g["bq0"][None]]).astype(f32) * isd)[PERM]
    Wk0a = np.vstack([g["Wk0"], g["bk0"][None]]).astype(f32)[PERM]
    A = np.stack([(Wk0a[:, h * D:(h + 1) * D] @ Wq0a[:, h * D:(h + 1) * D].T)
                  for h in range(H)])                                # [4, 11, 11]
    AT = A.transpose(0, 2, 1).transpose(1, 0, 2).copy()              # [11, 4, 11]

    kxaug = np.concatenate([kx, np.ones((K, N, 1), f32)], axis=2)[:, :, PERM]
    kxTaug = kxaug.transpose(0, 2, 1).copy()                         # [10, 11, 1024]

    ide = f32(1.0 / np.sqrt(DH))
    Wqkv, bqkv = g["Wqkv"].astype(f32), g["bqkv"].astype(f32)

    def w2t(w, nch):
        m = w.shape[1]
        return np.ascontiguousarray(
            np.asarray(w, f32).reshape(nch, P, m).transpose(1, 0, 2))

    Wv0a = np.vstack([g["Wv0"], g["bv0"][None]]).astype(f32)[PERM]   # [11, 1024]
    Wv0az = np.zeros((AUG, H, D + 1), f32)
    for h in range(H):
        Wv0az[:, h, 0:D] = Wv0a[:, h * D:(h + 1) * D]
    Wv0az[10, :, D] = 1.0      # e10 column (ones-row index under PERM) -> z

    common = {
        "AT": AT.astype(f16),
        "Wv0az": Wv0az.astype(f16),
        "Wr0a": np.vstack([g["Wr0"], g["br0"][None]])[PERM].astype(f16),
        "Wq1s": w2t(np.asarray(g["Wq1"], f32) * isd, 2).astype(f16),
        "Wk1": w2t(g["Wk1"], 2).astype(f16),
        "Wv1": w2t(g["Wv1"], 2).astype(f16),
        "bq1b": np.ascontiguousarray(np.broadcast_to(
            (np.asarray(g["bq1"], f32) * isd).reshape(8, P).T[:, :, None],
            (P, 8, P))).astype(f32),
        "bk1b": np.ascontiguousarray(np.broadcast_to(
            np.asarray(g["bk1"], f32).reshape(8, P).T[:, :, None],
            (P, 8, P))).astype(f32),
        "bv1": np.asarray(g["bv1"], f16)[None],
        "Wr1": w2t(g["Wr1"], 2).astype(f16),
        "brpe1": (np.asarray(g["br1"], f32)[None]
                  + np.asarray(g["pe"], f32))[:, None, :].astype(f16),
        "Wqe": w2t(Wqkv[:, 0:DM] * ide, 2).astype(f16),
        "Wke": w2t(Wqkv[:, DM:2 * DM], 2).astype(f16),
        "Wve": w2t(Wqkv[:, 2 * DM:], 2).astype(f16),
        "bqe": (bqkv[0:DM] * ide)[None].astype(f16),
        "bke": bqkv[DM:2 * DM][None].astype(f16),
        "bve": bqkv[2 * DM:][None].astype(f16),
        "Wo": w2t(g["Wo"], 2).astype(f16),
        "bo": np.asarray(g["bo"], f16)[None],
        "W1": w2t(g["W1"], 2).astype(f16),
        "b1f": np.asarray(g["b1f"], f16)[None],
        "W2": w2t(g["W2"], 16).astype(f16),
        "b2f": np.asarray(g["b2f"], f16)[None],
        "Wd": w2t(g["Wd"], 2).astype(f16),
        "bd": np.asarray(g["bd"], f16)[None],
        "g1": np.asarray(g["g1"], f32)[None],
        "be1": np.asarray(g["be1"], f32)[None],
        "g2": np.asarray(g["g2"], f32)[None],
        "be2": np.asarray(g["be2"], f32)[None],
        "idn": np.eye(P, dtype=f16),
        "ones1": np.ones((1, P), f16),
        "kxTaug_all": kxTaug.astype(f16),
        "kxaug_all": np.ascontiguousarray(
            kxaug.reshape(K, 8, P, AUG).transpose(0, 2, 1, 3)).astype(f16),
    }
    in_maps = []
    for c in range(NC_):
        sl = slice(P * c, P * (c + 1))
        m = dict(common)
        m["lt"] = np.ascontiguousarray(
            LT[:, sl].reshape(8, P, P).transpose(1, 0, 2)).astype(f16)
        m["latlon32"] = np.ascontiguousarray(kx[K - 1, sl, 0:2]).astype(f32)
        ll = kx[K - 1, sl, 0:2].astype(f32)
        xti = np.zeros((AUG, P), f32); xti[8:10] = ll.T; xti[10] = 1.0
        m["xTaug_init"] = xti.astype(f16)
        xai = np.zeros((P, AUG), f32); xai[:, 8:10] = ll; xai[:, 10] = 1.0
        m["xaug_init"] = xai.astype(f16)
        m["kxTaug_loc"] = np.ascontiguousarray(kxTaug[:, :, sl]).astype(f16)
        in_maps.append(m)
    return in_maps


_CACHED = {}


def run(inputs, trace=False, trace_kwargs=None):
    from concourse import bass_utils
    if "nc" not in _CACHED:
        _CACHED["nc"] = build_bass()
    in_maps = prep_in_maps(inputs)
    res = bass_utils.run_bass_kernel_spmd(
        _CACHED["nc"], in_maps, core_ids=list(range(NC_)), trace=trace,
        **(trace_kwargs or {}))
    out = np.concatenate([res.results[c]["out"] for c in range(NC_)], axis=1)
    return out.astype(np.float32), res


def kernel(**inputs):
    out, _ = run(inputs, trace=False)
    return out
